# revision 6
# baseline (speedup 1.0000x reference)
"""Trainium2 Bass kernel for the Conv2.5d depth-masked convolution problem.

Math (per batch b, output pixel (y,x), f scalar):
  d0 = depth[b,0,y,x]; s0 = d0/f
  For tap (i,j) in 3x3 window, dw = depth[b,0,y+i-1,x+j-1] (zero-padded):
    level l in {0,1,2} active iff  d0*(1+(l-1.5)/f) <= dw < d0*(1+(l-0.5)/f)
  out[b,o,y,x] = sum_{l,i,j,c} W[l,o,c,i,j] * inputs[b,c,y+i-1,x+j-1] * mask
                 + bias[o]

Fast-plan kernel strategy (8 NeuronCores, data-parallel over (batch, y-half)):
  - Telescoped weights V1=W1-W0, V2=W2-W1, V3=-W2 turn the 3 interval masks
    into 3 step masks g_k = [q >= c_k], q = dw/d0, plus an unmasked W0 term.
  - Sign reformulation: with s_k = sign(q - c_k) in {-1,+1},
      V_k*g_k = (V_k/2)*s_k + V_k/2, so
      out_tap = (W0/2) (x) S  +  sum_k (V_k/2) (x) (s_k * S).
    The s_k masks are produced on the otherwise-idle Scalar (ACT) engine via
    func=Sign with bias=-c_k; exactness holds because fp32 subtraction
    preserves the sign of q - c_k (no ties on this dataset, host-verified).
  - 4-tap x 32-channel partition layout: image and depth tap windows are
    pre-shifted into [4 taps x 32ch, 64x64] tiles, so ONE mask tile serves
    both channel-half multiplies, ONE Pool STT computes q for 4 taps, and
    each DVE bf16 tensor_tensor (2 elem/cycle mode) builds a full matmul rhs.
  - Engine split per chunk: PE 17 matmul groups (bf16), ACT 6 Sign masks,
    DVE 12 bf16 mults + reciprocal, Pool 2 q STTs + bias-add eviction.
  - Reciprocal via the 1-pass custom-DVE approx (bit-exactly emulated on the
    host); kernel() verifies on the host that the whole plan reproduces the
    reference masks for this dataset and falls back to the legacy exact-plan
    program otherwise.
"""

import numpy as np
import ml_dtypes

import concourse.mybir as mybir
from concourse import bacc
from concourse.tile import TileContext
from concourse.bass_utils import run_bass_kernel_spmd

# ---- problem constants (hardcoded per contest rules) ----
B, CIN, COUT, H, W = 4, 64, 64, 128, 128
KK = 3
N_CORES = 8
HY = H // 2               # rows per core (y-half)
SLAB_R, SLAB_C = 68, 132  # host padded slab (rows y0-1 .. y0+66, cols -1 .. 130)
CHUNK_Y = 16              # y-rows per psum chunk
CHUNK = CHUNK_Y * 64      # 1024 pixels per chunk
NSLICE = CHUNK // 512     # matmul free-dim slices per chunk
NCH = HY // CHUNK_Y       # chunks per x-half

G1 = [(0, 0), (0, 1), (0, 2), (1, 0)]   # 4-tap groups (center excluded)
G2 = [(1, 2), (2, 0), (2, 1), (2, 2)]
NG = 17                   # matmul groups: 4 raw + 1 center + 12 masked

BF16 = ml_dtypes.bfloat16

_CACHE = {}
TRACE = False            # set by test harness to collect an NTFF profile
LAST_EXEC_NS = None
LAST_PROFILE = None


# ---------------------------------------------------------------------------
# host-side helpers (fast plan)
# ---------------------------------------------------------------------------

def _recip_fast_host(x):
    """Bit-exact fp32 emulation of the custom-DVE RECIPROCAL_APPROX_FAST."""
    c0, c1, c2 = np.float32(-0.23549792), np.float32(2.0017324), np.float32(2.0)
    not_x = (~x.view(np.int32)).view(np.float32)
    y0 = (not_x * c0).astype(np.float32)
    y1 = (y0 * (c1 - (x * y0).astype(np.float32)).astype(np.float32)).astype(np.float32)
    return (y1 * (c2 - (x * y1).astype(np.float32)).astype(np.float32)).astype(np.float32)


def _fast_plan_safe(depth, cks):
    """Check that sign(fl(fl(dw*r0_fast) - c_k)) reproduces the reference
    masks for every non-center tap of this dataset (no flips, no ties), and
    that every center depth is strictly positive."""
    d0 = np.asarray(depth, np.float32)[:, 0]          # [B,H,W]
    if not (d0 > 0).all():
        return False
    dpad = np.zeros((B, H + 2, W + 2), np.float32)
    dpad[:, 1:-1, 1:-1] = d0
    r0 = _recip_fast_host(d0)
    if not np.isfinite(r0).all():
        return False
    for i in range(KK):
        for j in range(KK):
            if i == 1 and j == 1:
                continue
            dw = dpad[:, i:i + H, j:j + W]
            q = (dw * r0).astype(np.float32)
            for ck in cks:
                exact = dw >= (np.float32(ck) * d0).astype(np.float32)
                if not np.array_equal(exact, q >= np.float32(ck)):
                    return False
                if (q == np.float32(ck)).any():
                    return False    # Sign would give 0 at an exact tie
    return True


def _host_slabs_fast(inputs, depth):
    """Per-core padded slabs: img bf16 [64, 68*132], dep f32 [1, 68*132]."""
    inputs_bf = inputs.astype(BF16)
    Ih, Dh = [], []
    for b in range(B):
        for half in range(2):
            y0 = half * HY
            Islab = np.zeros((CIN, SLAB_R, SLAB_C), BF16)
            Dslab = np.zeros((SLAB_R, SLAB_C), np.float32)
            ylo, yhi = y0 - 1, y0 + SLAB_R - 1
            sy0, sy1 = max(ylo, 0), min(yhi, H)
            Islab[:, sy0 - ylo:sy1 - ylo, 1:1 + W] = inputs_bf[b, :, sy0:sy1, :]
            Dslab[sy0 - ylo:sy1 - ylo, 1:1 + W] = depth[b, 0, sy0:sy1, :]
            Ih.append(np.ascontiguousarray(Islab.reshape(CIN, -1)))
            Dh.append(np.ascontiguousarray(Dslab.reshape(1, -1)))
    return Ih, Dh


def _pack_weights_fast(weight):
    """lhsT tensors [128, NG*64] bf16.

    Group order: 0-3 raw (G1 ch-lo, G1 ch-hi, G2 ch-lo, G2 ch-hi), 4 center,
    5.. masked ((group, k, ch-half) nested). Raw weight = W0/2; masked
    weight = V_k/2 (ACT Sign masks are +-1)."""
    Wl = [np.asarray(weight[l], np.float32) for l in range(KK)]  # [O,C,3,3]
    V = [None, Wl[1] - Wl[0], Wl[2] - Wl[1], -Wl[2]]
    Wp = np.zeros((NG, 128, 64), np.float32)
    for gi, taps in enumerate((G1, G2)):
        for hi in range(2):
            g = gi * 2 + hi
            for ti, (i, j) in enumerate(taps):
                Wp[g, ti * 32:(ti + 1) * 32, :] = \
                    Wl[0][:, hi * 32:(hi + 1) * 32, i, j].T / 2
    Wp[4, 0:64, :] = Wl[1][:, :, 1, 1].T
    g = 5
    for gi, taps in enumerate((G1, G2)):
        for k in (1, 2, 3):
            for hi in range(2):
                for ti, (i, j) in enumerate(taps):
                    Wp[g, ti * 32:(ti + 1) * 32, :] = \
                        V[k][:, hi * 32:(hi + 1) * 32, i, j].T / 2
                g += 1
    assert g == NG
    Wp = Wp.astype(BF16)
    return np.ascontiguousarray(Wp.transpose(1, 0, 2).reshape(128, NG * 64))


def _build_fast(cks):
    nc = bacc.Bacc("TRN2", target_bir_lowering=False)
    f32, bf = mybir.dt.float32, mybir.dt.bfloat16
    img = nc.declare_dram_parameter("img", [CIN, SLAB_R * SLAB_C], bf, isOutput=False)
    dep = nc.declare_dram_parameter("dep", [1, SLAB_R * SLAB_C], f32, isOutput=False)
    wp = nc.declare_dram_parameter("wp", [128, NG * 64], bf, isOutput=False)
    bia = nc.declare_dram_parameter("bia", [COUT, 1], f32, isOutput=False)
    out = nc.declare_dram_parameter("out", [COUT, HY, W], f32, isOutput=True)

    mult, add = mybir.AluOpType.mult, mybir.AluOpType.add
    sign_fn = mybir.ActivationFunctionType.Sign

    with TileContext(nc) as tc:
        with tc.tile_pool(name="w", bufs=1) as wpool, \
             tc.tile_pool(name="img", bufs=2) as ipool, \
             tc.tile_pool(name="ddc", bufs=1) as cpool, \
             tc.tile_pool(name="r0", bufs=2) as rpool, \
             tc.tile_pool(name="dd", bufs=2) as dpool, \
             tc.tile_pool(name="q", bufs=2) as qpool, \
             tc.tile_pool(name="m", bufs=3) as mpool, \
             tc.tile_pool(name="x", bufs=4) as xpool, \
             tc.tile_pool(name="o", bufs=2) as opool, \
             tc.tile_pool(name="ps", bufs=2, space="PSUM") as pspool:

            wt = wpool.tile([128, NG * 64], bf)
            nc.sync.dma_start(out=wt[:], in_=wp[:, :])
            bt = wpool.tile([COUT, 1], f32)
            nc.sync.dma_start(out=bt[:], in_=bia[:, :])
            # per-level Sign biases (-c_k), memset once at startup
            ct = wpool.tile([128, 3], f32)
            for k in (1, 2, 3):
                nc.gpsimd.memset(ct[:, k - 1:k], -float(cks[k - 1]))

            def lhsT(g, k128=True):
                v = wt[:, g * 64:(g + 1) * 64]
                return v if k128 else wt[0:64, g * 64:(g + 1) * 64]

            img3 = img.rearrange("p (r c) -> p r c", r=SLAB_R)
            dep3 = dep.rearrange("p (r c) -> p r c", r=SLAB_R)

            for hx in range(2):
                cx = hx * 64

                # pre-shifted image tap tiles: [4 taps x 32ch, 64*64] bf16
                iit = {}
                for gi, taps in enumerate((G1, G2)):
                    for hi in range(2):
                        t4 = ipool.tile([128, 64 * 64], bf, tag=f"ii{gi}{hi}")
                        v = t4.rearrange("p (r c) -> p r c", r=64)
                        for ti, (i, j) in enumerate(taps):
                            nc.sync.dma_start(
                                out=v[ti * 32:(ti + 1) * 32],
                                in_=img3[hi * 32:(hi + 1) * 32,
                                         i:i + 64, cx + j:cx + j + 64])
                        iit[(gi, hi)] = t4
                iic = ipool.tile([64, 64 * 64], bf, tag="iic")
                nc.sync.dma_start(
                    out=iic.rearrange("p (r c) -> p r c", r=64),
                    in_=img3[0:64, 1:65, cx + 1:cx + 65])

                # center depth broadcast + fast reciprocal (whole x-half)
                ddc = cpool.tile([128, 64 * 64], f32, tag="ddc")
                nc.sync.dma_start(
                    out=ddc.rearrange("p (r c) -> p r c", r=64),
                    in_=dep3[:, 1:65, cx + 1:cx + 65].to_broadcast([128, 64, 64]))
                r0 = rpool.tile([128, 64 * 64], f32, tag="r0")
                nc.vector.reciprocal_approx_fast(out=r0[:, :], in_=ddc[:, :])
                r03 = r0.rearrange("p (r c) -> p r c", r=64)

                def iiv(gi, hi):
                    return iit[(gi, hi)].rearrange("p (r c) -> p r c", r=64)

                for ch in range(NCH):
                    ry = ch * CHUNK_Y

                    # per-chunk depth tap tiles [4 taps x 32rep, 16*64] f32
                    ddt = []
                    for gi, taps in enumerate((G1, G2)):
                        dtile = dpool.tile([128, CHUNK], f32, tag=f"dd{gi}")
                        dv = dtile.rearrange("p (r c) -> p r c", r=CHUNK_Y)
                        for ti, (i, j) in enumerate(taps):
                            nc.sync.dma_start(
                                out=dv[ti * 32:(ti + 1) * 32],
                                in_=dep3[:, ry + i:ry + i + CHUNK_Y,
                                         cx + j:cx + j + 64]
                                    .to_broadcast([32, CHUNK_Y, 64]))
                        ddt.append(dtile)

                    ps = pspool.tile([COUT, CHUNK], mybir.dt.float32)
                    psv = ps.rearrange("p (y x) -> p y x", y=CHUNK_Y)
                    mm_i = [0]

                    def mm(lh, rhs, s):
                        nc.tensor.matmul(
                            psv[:, s * 8:s * 8 + 8, :], lh, rhs,
                            start=(mm_i[0] < NSLICE),
                            stop=(mm_i[0] >= NG * NSLICE - NSLICE))
                        mm_i[0] += 1

                    # raw groups first: no mask dependency, keeps PE busy
                    for gi in range(2):
                        for hi in range(2):
                            for s in range(NSLICE):
                                mm(lhsT(gi * 2 + hi),
                                   iiv(gi, hi)[:, ry + s * 8:ry + s * 8 + 8, :], s)
                    civ = iic.rearrange("p (r c) -> p r c", r=64)
                    for s in range(NSLICE):
                        mm(lhsT(4, False), civ[:, ry + s * 8:ry + s * 8 + 8, :], s)

                    g = 5
                    for gi in range(2):
                        q = qpool.tile([128, CHUNK], f32, tag=f"q{gi}")
                        nc.gpsimd.tensor_tensor(
                            out=q.rearrange("p (y x) -> p y x", y=CHUNK_Y),
                            in0=ddt[gi].rearrange("p (y x) -> p y x", y=CHUNK_Y),
                            in1=r03[:, ry:ry + CHUNK_Y, :],
                            op=mult)
                        for k in (1, 2, 3):
                            m = mpool.tile([128, CHUNK], bf, tag="m")
                            nc.scalar.activation(
                                out=m[:, :], in_=q[:, :], func=sign_fn,
                                bias=ct[:, k - 1:k])
                            m3 = m.rearrange("p (y x) -> p y x", y=CHUNK_Y)
                            for hi in range(2):
                                x = xpool.tile([128, CHUNK], bf, tag="x")
                                nc.vector.tensor_tensor(
                                    out=x.rearrange("p (y x) -> p y x", y=CHUNK_Y),
                                    in0=m3,
                                    in1=iiv(gi, hi)[:, ry:ry + CHUNK_Y, :],
                                    op=mult)
                                for s in range(NSLICE):
                                    mm(lhsT(g), x[:, s * 512:s * 512 + 512], s)
                                g += 1
                    assert mm_i[0] == NG * NSLICE

                    # eviction with fused bias add (DVE; GPSIMD can't read PSUM)
                    ot = opool.tile([COUT, CHUNK], f32, tag="o")
                    nc.vector.tensor_scalar(
                        out=ot[:, :], in0=ps[:, :],
                        scalar1=bt[:, :], scalar2=None, op0=add)
                    nc.sync.dma_start(
                        out=out[:, ry:ry + CHUNK_Y, cx:cx + 64],
                        in_=ot.rearrange("p (y x) -> p y x", y=CHUNK_Y))

    nc.finalize()
    return nc


# ---------------------------------------------------------------------------
# legacy exact-plan fallback (previous kernel, f32r pair-stacked STT design)
# ---------------------------------------------------------------------------

LG_HXW = 66
LG_SLAB_F = LG_HXW * LG_HXW
LG_PAIRS = [
    ((0, 0), (0, 2), 2),
    ((1, 0), (1, 2), 2),
    ((2, 0), (2, 2), 2),
    ((0, 1), (2, 1), 2 * LG_HXW),
]


def _lg_pack_weights(weight):
    Wl = [np.asarray(weight[l], np.float32) for l in range(KK)]
    V = [Wl[0], Wl[1] - Wl[0], Wl[2] - Wl[1], -Wl[2]]
    Wp = np.zeros((18, 128, 64), np.float32)
    g = 0
    for (ta, tb, _delta) in LG_PAIRS:
        for k in range(4):
            Wp[g, 0:64, :] = V[k][:, :, ta[0], ta[1]].T
            Wp[g, 64:128, :] = V[k][:, :, tb[0], tb[1]].T
            g += 1
    Wp[16, 0:64, :] = Wl[1][:, :, 1, 1].T
    Wp[17, 0:64, :] = -Wl[1][:, :, 1, 1].T
    return Wp


def _lg_host_slabs(inputs, depth):
    Ih, Dh = [], []
    for b in range(B):
        for half in range(2):
            y0 = half * HY
            Islab = np.zeros((CIN, SLAB_R, SLAB_C), np.float32)
            Dslab = np.zeros((SLAB_R, SLAB_C), np.float32)
            ylo, yhi = y0 - 1, y0 + SLAB_R - 1
            sy0, sy1 = max(ylo, 0), min(yhi, H)
            Islab[:, sy0 - ylo:sy1 - ylo, 1:1 + W] = inputs[b, :, sy0:sy1, :]
            Dslab[sy0 - ylo:sy1 - ylo, 1:1 + W] = depth[b, 0, sy0:sy1, :]
            Ih.append(np.ascontiguousarray(Islab.reshape(CIN, -1)))
            Dh.append(np.ascontiguousarray(Dslab.reshape(1, -1)))
    return Ih, Dh


def _build_legacy(cks):
    nc = bacc.Bacc("TRN2", target_bir_lowering=False)
    f32, f32r = mybir.dt.float32, mybir.dt.float32r
    img = nc.declare_dram_parameter("img", [CIN, SLAB_R * SLAB_C], f32, isOutput=False)
    dep = nc.declare_dram_parameter("dep", [1, SLAB_R * SLAB_C], f32, isOutput=False)
    wp = nc.declare_dram_parameter("wp", [128, 18 * 64], f32, isOutput=False)
    bia = nc.declare_dram_parameter("bia", [COUT, 1], f32, isOutput=False)
    out = nc.declare_dram_parameter("out", [COUT, HY, W], f32, isOutput=True)

    le, mult = mybir.AluOpType.is_le, mybir.AluOpType.mult

    with TileContext(nc) as tc:
        with tc.tile_pool(name="w", bufs=1) as wpool, \
             tc.tile_pool(name="slab", bufs=1) as spool, \
             tc.tile_pool(name="work", bufs=2) as qpool, \
             tc.tile_pool(name="xw", bufs=4) as xpool, \
             tc.tile_pool(name="ow", bufs=2) as opool, \
             tc.tile_pool(name="psum", bufs=2, space="PSUM") as pspool:

            wt = wpool.tile([128, 18 * 64], f32r)
            nc.gpsimd.dma_start(out=wt[:], in_=wp[:, :])
            bt = wpool.tile([COUT, 1], f32)
            nc.sync.dma_start(out=bt[:], in_=bia[:, :])

            def lhsT(g, k128=True):
                v = wt[:, g * 64:(g + 1) * 64]
                return v if k128 else wt[0:64, g * 64:(g + 1) * 64]

            for hx in range(2):
                cx = hx * 64

                def hsrc(t, roff, coff):
                    t3 = t.rearrange("p (r c) -> p r c", r=SLAB_R)
                    return t3[:, roff:roff + LG_HXW, cx + coff:cx + coff + LG_HXW]

                ii2 = spool.tile([128, LG_SLAB_F], f32r, tag="ii2")
                nc.gpsimd.dma_start(out=ii2[0:64, :].rearrange("p (r c) -> p r c", r=LG_HXW), in_=hsrc(img, 0, 0))
                nc.gpsimd.dma_start(out=ii2[64:128, :].rearrange("p (r c) -> p r c", r=LG_HXW), in_=hsrc(img, 0, 2))
                ii132 = spool.tile([128, LG_SLAB_F], f32r, tag="ii132")
                nc.gpsimd.dma_start(out=ii132[0:64, :].rearrange("p (r c) -> p r c", r=LG_HXW), in_=hsrc(img, 0, 0))
                nc.gpsimd.dma_start(out=ii132[64:128, :].rearrange("p (r c) -> p r c", r=LG_HXW), in_=hsrc(img, 2, 0))
                dd2 = spool.tile([128, LG_SLAB_F], f32, tag="dd2")
                nc.sync.dma_start(out=dd2[0:64, :].rearrange("p (r c) -> p r c", r=LG_HXW),
                                  in_=hsrc(dep, 0, 0).to_broadcast([64, LG_HXW, LG_HXW]))
                nc.sync.dma_start(out=dd2[64:128, :].rearrange("p (r c) -> p r c", r=LG_HXW),
                                  in_=hsrc(dep, 0, 2).to_broadcast([64, LG_HXW, LG_HXW]))
                dd132 = spool.tile([128, LG_SLAB_F], f32, tag="dd132")
                nc.sync.dma_start(out=dd132[0:64, :].rearrange("p (r c) -> p r c", r=LG_HXW),
                                  in_=hsrc(dep, 0, 0).to_broadcast([64, LG_HXW, LG_HXW]))
                nc.sync.dma_start(out=dd132[64:128, :].rearrange("p (r c) -> p r c", r=LG_HXW),
                                  in_=hsrc(dep, 2, 0).to_broadcast([64, LG_HXW, LG_HXW]))

                dd2v = dd2.rearrange("p (r c) -> p r c", r=LG_HXW)
                dd132v = dd132.rearrange("p (r c) -> p r c", r=LG_HXW)
                ii2v = ii2.rearrange("p (r c) -> p r c", r=LG_HXW)
                ii132v = ii132.rearrange("p (r c) -> p r c", r=LG_HXW)

                dc = spool.tile([128, 64 * 64], f32, tag="r0")
                nc.sync.dma_start(out=dc[0:64, :].rearrange("p (y x) -> p y x", y=64),
                                  in_=hsrc(dep, 1, 1)[:, 0:64, 0:64].to_broadcast([64, 64, 64]))
                nc.sync.dma_start(out=dc[64:128, :], in_=dc[0:64, :])

                for ch in range(HY // CHUNK_Y):
                    ry = ch * CHUNK_Y

                    def tapv(base3, tap, rows=CHUNK_Y, s=0):
                        i, j = tap
                        rr = i + ry
                        return base3[:, rr + s * 8:rr + s * 8 + rows, j:j + 64]

                    def centv(t, rows=CHUNK_Y, s=0):
                        v3 = t[:, :].rearrange("p (y x) -> p y x", y=64)
                        return v3[:, ry + s * 8:ry + s * 8 + rows, :]

                    ps = pspool.tile([COUT, CHUNK], mybir.dt.float32)
                    psv = ps.rearrange("p (y x) -> p y x", y=CHUNK_Y)
                    mm_i = [0]

                    def mm(lh, rhs, s):
                        nc.tensor.matmul(
                            psv[:, s * 8:s * 8 + 8, :], lh, rhs,
                            start=(mm_i[0] < NSLICE), stop=(mm_i[0] >= 18 * NSLICE - NSLICE))
                        mm_i[0] += 1

                    for p_i, (ta, tb, delta) in enumerate(LG_PAIRS):
                        ddv = dd2v if delta == 2 else dd132v
                        iiv = ii2v if delta == 2 else ii132v
                        g0 = p_i * 4
                        for s in range(NSLICE):
                            mm(lhsT(g0), tapv(iiv, ta, 8, s=s), s)
                        for k in (1, 2, 3):
                            gk = qpool.tile([128, CHUNK], f32, tag="q")
                            nc.vector.scalar_tensor_tensor(
                                out=gk.rearrange("p (y x) -> p y x", y=CHUNK_Y),
                                in0=centv(dc), scalar=float(cks[k - 1]),
                                in1=tapv(ddv, ta), op0=mult, op1=le)
                            x = xpool.tile([128, CHUNK], f32r, tag="x")
                            nc.vector.tensor_tensor(
                                out=x.rearrange("p (y x) -> p y x", y=CHUNK_Y),
                                in0=gk.rearrange("p (y x) -> p y x", y=CHUNK_Y),
                                in1=tapv(iiv, ta).bitcast(f32),
                                op=mybir.AluOpType.mult)
                            for s in range(NSLICE):
                                mm(lhsT(g0 + k), x[:, s * 512:s * 512 + 512], s)

                    for s in range(NSLICE):
                        mm(lhsT(16, False), tapv(ii2v[0:64], (1, 1), 8, s=s), s)
                    zm = qpool.tile([64, CHUNK], f32, tag="zm")
                    nc.vector.scalar_tensor_tensor(
                        out=zm.rearrange("p (y x) -> p y x", y=CHUNK_Y),
                        in0=tapv(dd2v[0:64], (1, 1)), scalar=float(cks[1]),
                        in1=tapv(dd2v[0:64], (1, 1)), op0=mult, op1=le)
                    xz = xpool.tile([64, CHUNK], f32r, tag="x")
                    nc.vector.tensor_tensor(
                        out=xz.rearrange("p (y x) -> p y x", y=CHUNK_Y),
                        in0=zm.rearrange("p (y x) -> p y x", y=CHUNK_Y),
                        in1=tapv(ii2v[0:64], (1, 1)).bitcast(f32),
                        op=mybir.AluOpType.mult)
                    for s in range(NSLICE):
                        mm(lhsT(17, False), xz[:, s * 512:s * 512 + 512], s)
                    assert mm_i[0] == 18 * NSLICE

                    ot = opool.tile([COUT, CHUNK], f32, tag="o")
                    nc.scalar.activation(
                        out=ot[:], in_=ps[:],
                        func=mybir.ActivationFunctionType.Identity, bias=bt[:])
                    nc.sync.dma_start(
                        out=out[:, ry:ry + CHUNK_Y, hx * 64:hx * 64 + 64],
                        in_=ot[:].rearrange("p (y x) -> p y x", y=CHUNK_Y))

    nc.finalize()
    return nc


# ---------------------------------------------------------------------------
# entry point
# ---------------------------------------------------------------------------

def kernel(inputs, depth, weight, bias, f):
    inputs = np.ascontiguousarray(np.asarray(inputs, np.float32))
    depth = np.ascontiguousarray(np.asarray(depth, np.float32))
    weight = np.asarray(weight, np.float32)
    bias_np = np.asarray(bias, np.float32).reshape(COUT, 1)
    fv = float(np.asarray(f).item() if hasattr(f, "item") or isinstance(f, np.ndarray) else f)
    # threshold coefficients c_k = 1 + (k - 1.5)/f, k = 1..3
    cks = [np.float32(1.0 + (k - 1.5) / fv) for k in (1, 2, 3)]
    assert 1.0 - 1.5 / fv <= 0.0, "f too large for the g0==1 simplification"

    fast = _fast_plan_safe(depth, cks)
    key = ("prog", tuple(np.float64(c) for c in cks), fast)
    if key not in _CACHE:
        _CACHE[key] = _build_fast(cks) if fast else _build_legacy(cks)
    nc = _CACHE[key]

    if fast:
        Ih, Dh = _host_slabs_fast(inputs, depth)
        Wp = _pack_weights_fast(weight)
    else:
        Ih, Dh = _lg_host_slabs(inputs, depth)
        Wp = np.ascontiguousarray(
            _lg_pack_weights(weight).transpose(1, 0, 2).reshape(128, 18 * 64))
    in_maps = [
        {"img": Ih[c], "dep": Dh[c], "wp": Wp, "bia": bias_np}
        for c in range(N_CORES)
    ]
    global LAST_EXEC_NS, LAST_PROFILE
    res = run_bass_kernel_spmd(nc, in_maps, list(range(N_CORES)), trace=TRACE)
    if TRACE:
        LAST_EXEC_NS = res.exec_time_ns
        LAST_PROFILE = res.profile_json
    outs = [res.results[c]["out"] for c in range(N_CORES)]
    full = np.empty((B, COUT, H, W), np.float32)
    for b in range(B):
        full[b, :, 0:HY, :] = outs[2 * b]
        full[b, :, HY:H, :] = outs[2 * b + 1]
    return full
